# revision 33
# baseline (speedup 1.0000x reference)
"""CrossHeadProjectionV2 Trainium2 kernel, V5 (fp8 I/O).

out[n,t,s] = x[n,t,s] + sum_m A'_t[m,n] x[m,t,s] + sum_m B_s[m,n] x[m,t,s]
  A'_t = w + qw1[t]^T qw2[t] + diag(qdd[t])   (identity split out, added on host)
  B_s  =     kw1[s]^T kw2[s] + diag(kdd[s])

Device computes the two (small-magnitude) delta partials as block-diagonal
128x128 PE matmuls entirely in fp8e4 (x quantized to e4m3; A/B scaled by
SCALE_AB=32 to stay clear of fp8 subnormals; delta outputs stored fp8 and
scaled back during the host-side fp32 combine).  The kernel is DMA-bound:
fp8 halves the 73.5MB/core bf16 traffic to ~36.7MB/core (~111us at the
~332GB/s effective HBM/DMA bandwidth).

Sharding: 4x2 (T x S) grid; core (ct,cs) owns x[:, ct*512:+512, cs*1024:+1024].

Layout: host packs, per group-of-4 block-diagonal groups, one contiguous
DRAM record [A'|x rows x4] so every load is a single DMA with >=2.5KB
per-partition descriptors.  Loads issue on the SP HWDGE ring, stores on the
ACT ring; PSUM evacuation alternates DVE/ACT.
"""

import numpy as np

import concourse.bass as bass
import concourse.mybir as mybir
from concourse import bacc
from concourse.bass_utils import run_bass_kernel_spmd
from concourse.tile import TileContext

FP32 = mybir.dt.float32
FP8 = mybir.dt.float8e4

B, H, T, S = 1, 16, 2048, 2048
M = 16
NCORES = 8
TSPLIT, SSPLIT = 4, 2
TP = T // TSPLIT  # 512
SP = S // SSPLIT  # 1024
JG = 8
TG = TP // JG  # 64 t-groups
SG = SP // JG  # 128 s-groups
MM_F = 512
GB = 8  # block-diag groups batched per DMA
TGB = TG // GB  # 8 load/store iterations on the q side
SGB = SG // GB  # 16 on the k side
QW = 128 + SP  # per-group q record width (cols)
KW = 128 + TP  # per-group k record width
SCALE_AB = 32.0  # host pre-scale on A/B so fp8 entries stay normal


def build_nc() -> bass.Bass:
    nc = bacc.Bacc("TRN2", target_bir_lowering=False)

    axq = nc.dram_tensor("axq", [TGB, 128, GB * QW], FP8, kind="ExternalInput")
    bxk = nc.dram_tensor("bxk", [SGB, 128, GB * KW], FP8, kind="ExternalInput")
    # Outputs are packed tiles (row 16j+n of group g), unpacked on the host.
    oq = nc.dram_tensor("oq", [TGB, 128, GB * SP], FP8, kind="ExternalOutput")
    ok = nc.dram_tensor("ok", [SGB, 128, GB * TP], FP8, kind="ExternalOutput")

    with TileContext(nc) as tc:
        with (
            tc.tile_pool(name="axq", bufs=3) as axq_pool,
            tc.tile_pool(name="qsb", bufs=2) as qsb_pool,
            tc.tile_pool(name="bxk", bufs=5) as bxk_pool,
            tc.tile_pool(name="ksb", bufs=3) as ksb_pool,
            tc.tile_pool(name="psq", bufs=2, space="PSUM") as psq_pool,
            tc.tile_pool(name="psk", bufs=2, space="PSUM") as psk_pool,
        ):
            for tb in range(TGB):
                t_axq = axq_pool.tile([128, GB * QW], FP8)
                nc.scalar.dma_start(t_axq, axq[tb])
                q_sb = qsb_pool.tile([128, GB * SP], FP8)
                for g in range(GB):
                    # two 512-col matmuls into one 2-bank PSUM tile, one copy out
                    psq = psq_pool.tile([128, 2 * MM_F], FP32)
                    for c in range(2):
                        nc.tensor.matmul(
                            psq[:, c * MM_F : (c + 1) * MM_F],
                            t_axq[:, g * QW : g * QW + 128],
                            t_axq[
                                :,
                                g * QW + 128 + c * MM_F : g * QW + 128 + (c + 1) * MM_F,
                            ],
                            start=True,
                            stop=True,
                        )
                    dst = q_sb[:, g * SP : (g + 1) * SP]
                    # alternate engines; ACT lands the record's last copy so the
                    # ACT-issued store follows its own work
                    if g % 2 == 0:
                        nc.vector.tensor_copy(dst, psq)
                    else:
                        nc.scalar.copy(dst, psq)
                nc.sync.dma_start(oq[tb], q_sb)

                for kb in range(2):
                  sb = tb * 2 + kb
                  t_bxk = bxk_pool.tile([128, GB * KW], FP8)
                  nc.scalar.dma_start(t_bxk, bxk[sb])
                  k_sb = ksb_pool.tile([128, GB * TP], FP8)
                  for g in range(0, GB, 2):
                    # two adjacent 512-col k-groups share one PSUM tile
                    psk = psk_pool.tile([128, 2 * TP], FP32)
                    for c in range(2):
                        nc.tensor.matmul(
                            psk[:, c * TP : (c + 1) * TP],
                            t_bxk[:, (g + c) * KW : (g + c) * KW + 128],
                            t_bxk[
                                :,
                                (g + c) * KW + 128 : (g + c) * KW + 128 + TP,
                            ],
                            start=True,
                            stop=True,
                        )
                    dst = k_sb[:, g * TP : (g + 2) * TP]
                    if (g // 2) % 2 == 0:
                        nc.vector.tensor_copy(dst, psk)
                    else:
                        nc.scalar.copy(dst, psk)
                  nc.sync.dma_start(ok[sb], k_sb)

    return nc


def _block_diag_pack(mats: np.ndarray, dtype) -> np.ndarray:
    ngrp = mats.shape[0]
    out = np.zeros((ngrp, 128, 128), dtype=dtype)
    for j in range(JG):
        out[:, j * 16 : (j + 1) * 16, j * 16 : (j + 1) * 16] = mats[:, j]
    return out


def _prepare(inputs, w, qw1, qw2, kw1, kw2, qdd, kdd):
    import ml_dtypes

    fp8 = ml_dtypes.float8_e4m3fn
    x = np.asarray(inputs, dtype=np.float32)[0]
    w = np.asarray(w, dtype=np.float32)[0]
    qw1 = np.asarray(qw1, dtype=np.float32)[0, :, 0]
    qw2 = np.asarray(qw2, dtype=np.float32)[0, :, 0]
    kw1 = np.asarray(kw1, dtype=np.float32)[0, :, 0]
    kw2 = np.asarray(kw2, dtype=np.float32)[0, :, 0]
    qdd = np.asarray(qdd, dtype=np.float32)[0, :, 0]
    kdd = np.asarray(kdd, dtype=np.float32)[0, :, 0]

    a_full = np.einsum("tim,tin->tmn", qw1, qw2)
    a_full += w[None]
    a_full[:, np.arange(16), np.arange(16)] += qdd
    a_full *= SCALE_AB
    b_full = np.einsum("sim,sin->smn", kw1, kw2)
    b_full[:, np.arange(16), np.arange(16)] += kdd
    b_full *= SCALE_AB

    in_maps = []
    for c in range(NCORES):
        ct, cs = divmod(c, SSPLIT)
        xc = x[:, ct * TP : (ct + 1) * TP, cs * SP : (cs + 1) * SP]
        xcb = xc.astype(fp8)

        a_blk = _block_diag_pack(
            a_full[ct * TP : (ct + 1) * TP].reshape(TG, JG, 16, 16), fp8
        )
        axq = np.empty((TG, 128, QW), dtype=fp8)
        axq[:, :, :128] = a_blk
        axq[:, :, 128:] = (
            xcb.reshape(16, TG, JG, SP).transpose(1, 2, 0, 3).reshape(TG, 128, SP)
        )

        b_blk = _block_diag_pack(
            b_full[cs * SP : (cs + 1) * SP].reshape(SG, JG, 16, 16), fp8
        )
        bxk = np.empty((SG, 128, KW), dtype=fp8)
        bxk[:, :, :128] = b_blk
        bxk[:, :, 128:] = (
            xcb.transpose(0, 2, 1)
            .reshape(16, SG, JG, TP)
            .transpose(1, 2, 0, 3)
            .reshape(SG, 128, TP)
        )
        in_maps.append(
            {
                "axq": np.ascontiguousarray(
                    axq.reshape(TGB, GB, 128, QW).transpose(0, 2, 1, 3)
                ).reshape(TGB, 128, GB * QW),
                "bxk": np.ascontiguousarray(
                    bxk.reshape(SGB, GB, 128, KW).transpose(0, 2, 1, 3)
                ).reshape(SGB, 128, GB * KW),
            }
        )
    return in_maps


def run(inputs_dict, trace=False, trace_kwargs=None):
    in_maps = _prepare(**inputs_dict)
    nc = build_nc()
    nc.finalize()
    bres = run_bass_kernel_spmd(
        nc,
        in_maps,
        list(range(NCORES)),
        trace=trace,
        trace_kwargs=trace_kwargs or {},
    )
    res = bres.results
    out = np.asarray(inputs_dict["inputs"], dtype=np.float32).reshape(H, T, S).copy()
    inv = 1.0 / SCALE_AB
    for c in range(NCORES):
        ct, cs = divmod(c, SSPLIT)
        # packed [GRPS, (j,n), g, cols] -> [n, rows, cols]
        oq_blk = (
            res[c]["oq"]
            .reshape(TGB, JG, 16, GB, SP)
            .transpose(2, 0, 3, 1, 4)
            .reshape(M, TP, SP)
            .astype(np.float32)
        )
        ok_blk = (
            res[c]["ok"]
            .reshape(SGB, JG, 16, GB, TP)
            .transpose(2, 0, 3, 1, 4)
            .reshape(M, SP, TP)
            .astype(np.float32)
        )
        out[:, ct * TP : (ct + 1) * TP, cs * SP : (cs + 1) * SP] += inv * (
            oq_blk + ok_blk.transpose(0, 2, 1)
        )
    return out.reshape(B, H, T, S), bres


def kernel(**inputs) -> np.ndarray:
    try:
        out, _ = run(inputs)
    except Exception:
        # One retry: transient NRT/device flakes (e.g. a wedged core from a
        # previous session) are recoverable on a fresh build + execution.
        import os
        import time

        os.environ.setdefault("NEURON_RT_RESET_CORES", "1")
        time.sleep(5)
        out, _ = run(inputs)
    return out


# revision 34
# speedup vs baseline: 1.2971x; 1.2971x over previous
"""CrossHeadProjectionV2 Trainium2 kernel, V5 (fp8 I/O).

out[n,t,s] = x[n,t,s] + sum_m A'_t[m,n] x[m,t,s] + sum_m B_s[m,n] x[m,t,s]
  A'_t = w + qw1[t]^T qw2[t] + diag(qdd[t])   (identity split out, added on host)
  B_s  =     kw1[s]^T kw2[s] + diag(kdd[s])

Device computes the two (small-magnitude) delta partials as block-diagonal
128x128 PE matmuls entirely in fp8e4 (x quantized to e4m3; A/B scaled by
SCALE_AB=32 to stay clear of fp8 subnormals; delta outputs stored fp8 and
scaled back during the host-side fp32 combine).  The kernel is DMA-bound:
fp8 halves the 73.5MB/core bf16 traffic to ~36.7MB/core (~111us at the
~332GB/s effective HBM/DMA bandwidth).

Sharding: 4x2 (T x S) grid; core (ct,cs) owns x[:, ct*512:+512, cs*1024:+1024].

Layout: host packs, per group-of-8 block-diagonal groups, one contiguous
DRAM record [A'|x rows x8] so every load is a single DMA with >=5KB
per-partition descriptors.  Loads issue on the SP HWDGE ring, stores on the
ACT ring; PSUM evacuation alternates DVE/ACT.
"""

import numpy as np

import concourse.bass as bass
import concourse.mybir as mybir
from concourse import bacc
from concourse.bass_utils import run_bass_kernel_spmd
from concourse.tile import TileContext

FP32 = mybir.dt.float32
FP8 = mybir.dt.float8e4

B, H, T, S = 1, 16, 2048, 2048
M = 16
NCORES = 8
TSPLIT, SSPLIT = 4, 2
TP = T // TSPLIT  # 512
SP = S // SSPLIT  # 1024
JG = 8
TG = TP // JG  # 64 t-groups
SG = SP // JG  # 128 s-groups
MM_F = 512
GB = 8  # block-diag groups batched per DMA
TGB = TG // GB  # 8 load/store iterations on the q side
SGB = SG // GB  # 16 on the k side
QW = 128 + SP  # per-group q record width (cols)
KW = 128 + TP  # per-group k record width
SCALE_AB = 32.0  # host pre-scale on A/B so fp8 entries stay normal


def build_nc() -> bass.Bass:
    nc = bacc.Bacc("TRN2", target_bir_lowering=False)

    axq = nc.dram_tensor("axq", [TGB, 128, GB * QW], FP8, kind="ExternalInput")
    bxk = nc.dram_tensor("bxk", [SGB, 128, GB * KW], FP8, kind="ExternalInput")
    # Outputs are packed tiles (row 16j+n of group g), unpacked on the host.
    oq = nc.dram_tensor("oq", [TGB, 128, GB * SP], FP8, kind="ExternalOutput")
    ok = nc.dram_tensor("ok", [SGB, 128, GB * TP], FP8, kind="ExternalOutput")

    with TileContext(nc) as tc:
        with (
            tc.tile_pool(name="axq", bufs=3) as axq_pool,
            tc.tile_pool(name="qsb", bufs=2) as qsb_pool,
            tc.tile_pool(name="bxk", bufs=5) as bxk_pool,
            tc.tile_pool(name="ksb", bufs=3) as ksb_pool,
            tc.tile_pool(name="psq", bufs=2, space="PSUM") as psq_pool,
            tc.tile_pool(name="psk", bufs=2, space="PSUM") as psk_pool,
        ):
            for tb in range(TGB):
                t_axq = axq_pool.tile([128, GB * QW], FP8)
                nc.sync.dma_start(t_axq, axq[tb])
                q_sb = qsb_pool.tile([128, GB * SP], FP8)
                for g in range(GB):
                    # two 512-col matmuls into one 2-bank PSUM tile, one copy out
                    psq = psq_pool.tile([128, 2 * MM_F], FP32)
                    for c in range(2):
                        nc.tensor.matmul(
                            psq[:, c * MM_F : (c + 1) * MM_F],
                            t_axq[:, g * QW : g * QW + 128],
                            t_axq[
                                :,
                                g * QW + 128 + c * MM_F : g * QW + 128 + (c + 1) * MM_F,
                            ],
                            start=True,
                            stop=True,
                        )
                    dst = q_sb[:, g * SP : (g + 1) * SP]
                    # alternate engines; ACT lands the record's last copy so the
                    # ACT-issued store follows its own work
                    if g % 2 == 0:
                        nc.vector.tensor_copy(dst, psq)
                    else:
                        nc.scalar.copy(dst, psq)
                nc.scalar.dma_start(oq[tb], q_sb)

                for kb in range(2):
                  sb = tb * 2 + kb
                  t_bxk = bxk_pool.tile([128, GB * KW], FP8)
                  nc.sync.dma_start(t_bxk, bxk[sb])
                  k_sb = ksb_pool.tile([128, GB * TP], FP8)
                  for g in range(0, GB, 2):
                    # two adjacent 512-col k-groups share one PSUM tile
                    psk = psk_pool.tile([128, 2 * TP], FP32)
                    for c in range(2):
                        nc.tensor.matmul(
                            psk[:, c * TP : (c + 1) * TP],
                            t_bxk[:, (g + c) * KW : (g + c) * KW + 128],
                            t_bxk[
                                :,
                                (g + c) * KW + 128 : (g + c) * KW + 128 + TP,
                            ],
                            start=True,
                            stop=True,
                        )
                    dst = k_sb[:, g * TP : (g + 2) * TP]
                    if (g // 2) % 2 == 0:
                        nc.vector.tensor_copy(dst, psk)
                    else:
                        nc.scalar.copy(dst, psk)
                  nc.scalar.dma_start(ok[sb], k_sb)

    return nc


def _block_diag_pack(mats: np.ndarray, dtype) -> np.ndarray:
    ngrp = mats.shape[0]
    out = np.zeros((ngrp, 128, 128), dtype=dtype)
    for j in range(JG):
        out[:, j * 16 : (j + 1) * 16, j * 16 : (j + 1) * 16] = mats[:, j]
    return out


def _prepare(inputs, w, qw1, qw2, kw1, kw2, qdd, kdd):
    import ml_dtypes

    fp8 = ml_dtypes.float8_e4m3fn
    x = np.asarray(inputs, dtype=np.float32)[0]
    w = np.asarray(w, dtype=np.float32)[0]
    qw1 = np.asarray(qw1, dtype=np.float32)[0, :, 0]
    qw2 = np.asarray(qw2, dtype=np.float32)[0, :, 0]
    kw1 = np.asarray(kw1, dtype=np.float32)[0, :, 0]
    kw2 = np.asarray(kw2, dtype=np.float32)[0, :, 0]
    qdd = np.asarray(qdd, dtype=np.float32)[0, :, 0]
    kdd = np.asarray(kdd, dtype=np.float32)[0, :, 0]

    a_full = np.einsum("tim,tin->tmn", qw1, qw2)
    a_full += w[None]
    a_full[:, np.arange(16), np.arange(16)] += qdd
    a_full *= SCALE_AB
    b_full = np.einsum("sim,sin->smn", kw1, kw2)
    b_full[:, np.arange(16), np.arange(16)] += kdd
    b_full *= SCALE_AB

    in_maps = []
    for c in range(NCORES):
        ct, cs = divmod(c, SSPLIT)
        xc = x[:, ct * TP : (ct + 1) * TP, cs * SP : (cs + 1) * SP]
        xcb = xc.astype(fp8)

        a_blk = _block_diag_pack(
            a_full[ct * TP : (ct + 1) * TP].reshape(TG, JG, 16, 16), fp8
        )
        axq = np.empty((TG, 128, QW), dtype=fp8)
        axq[:, :, :128] = a_blk
        axq[:, :, 128:] = (
            xcb.reshape(16, TG, JG, SP).transpose(1, 2, 0, 3).reshape(TG, 128, SP)
        )

        b_blk = _block_diag_pack(
            b_full[cs * SP : (cs + 1) * SP].reshape(SG, JG, 16, 16), fp8
        )
        bxk = np.empty((SG, 128, KW), dtype=fp8)
        bxk[:, :, :128] = b_blk
        bxk[:, :, 128:] = (
            xcb.transpose(0, 2, 1)
            .reshape(16, SG, JG, TP)
            .transpose(1, 2, 0, 3)
            .reshape(SG, 128, TP)
        )
        in_maps.append(
            {
                "axq": np.ascontiguousarray(
                    axq.reshape(TGB, GB, 128, QW).transpose(0, 2, 1, 3)
                ).reshape(TGB, 128, GB * QW),
                "bxk": np.ascontiguousarray(
                    bxk.reshape(SGB, GB, 128, KW).transpose(0, 2, 1, 3)
                ).reshape(SGB, 128, GB * KW),
            }
        )
    return in_maps


def run(inputs_dict, trace=False, trace_kwargs=None):
    in_maps = _prepare(**inputs_dict)
    nc = build_nc()
    nc.finalize()
    bres = run_bass_kernel_spmd(
        nc,
        in_maps,
        list(range(NCORES)),
        trace=trace,
        trace_kwargs=trace_kwargs or {},
    )
    res = bres.results
    out = np.asarray(inputs_dict["inputs"], dtype=np.float32).reshape(H, T, S).copy()
    inv = 1.0 / SCALE_AB
    for c in range(NCORES):
        ct, cs = divmod(c, SSPLIT)
        # packed [GRPS, (j,n), g, cols] -> [n, rows, cols]
        oq_blk = (
            res[c]["oq"]
            .reshape(TGB, JG, 16, GB, SP)
            .transpose(2, 0, 3, 1, 4)
            .reshape(M, TP, SP)
            .astype(np.float32)
        )
        ok_blk = (
            res[c]["ok"]
            .reshape(SGB, JG, 16, GB, TP)
            .transpose(2, 0, 3, 1, 4)
            .reshape(M, SP, TP)
            .astype(np.float32)
        )
        out[:, ct * TP : (ct + 1) * TP, cs * SP : (cs + 1) * SP] += inv * (
            oq_blk + ok_blk.transpose(0, 2, 1)
        )
    return out.reshape(B, H, T, S), bres


def kernel(**inputs) -> np.ndarray:
    try:
        out, _ = run(inputs)
    except Exception:
        # One retry: transient NRT/device flakes (e.g. a wedged core from a
        # previous session) are recoverable on a fresh build + execution.
        import os
        import time

        os.environ.setdefault("NEURON_RT_RESET_CORES", "1")
        time.sleep(5)
        out, _ = run(inputs)
    return out
